# revision 3
# baseline (speedup 1.0000x reference)
"""Trainium2 Bass kernel for nn_LoRAElementLinear (MoE-routed per-node linear).

Math (reference):
    delta_w[z] = lora_A[z] contracted with lora_B[z] * SCALING         # [OUT, IN]
    W[z]       = (weights[z] + delta_w[z]) * ALPHA                     # [OUT, IN]
    out[b]     = sum_z node_attrs[b, z] * (W[z] @ t[b])                # [OUT, M]

node_attrs is a one-hot expert indicator (moe_routing), so out[b] = W[expert(b)] @ t[b].
The LoRA merge is folded into W on the host; the device runs only the routed
batched matmul.

Numerics: t is shipped as fp8 e3m4 (x2 pre-scale, folded back out through a
0.5x on W), W as bf16, PSUM accumulates fp32, output bf16. Measured on the
real data (host fp64 sim): rel err 1.30e-2 vs the 2e-2 gate. e4m3 anywhere
measures 2.6-3.9e-2 (fails), both-e3m4 measures 1.96e-2 (too close), so
DoubleRow fp8 (e4m3-only) is unusable and the PE runs at the bf16 rate of
1 col/cycle; fp8 on t is purely a DMA-bytes win (t is the largest input).

Sharding (host side): group nodes by expert. The two LARGEST experts are
split into 4 quarter-pieces each (cores 0-3 carry quarters of the largest,
cores 4-7 of the second); the remaining eight experts go whole, one per
core. Per-core profile [s1=max(whole counts), s2=max quarter] is SPMD-
uniform. On the seed-0 data: s1=828, s2=220 -> 1048 slots/core (2.3% pad)
vs the old split-the-two-smallest scheme's 1100 (7.4% pad).

Per-core streams (s1=828, s2=220 -> 3144 cols):
  - PE: 16 passes x 3144 cols ~ 50.3k cycles ~ 22 us warm.
  - DMA: t fp8 1.61 MB + W bf16 1.05 MB + out bf16 3.22 MB = 5.9 MB
    (~19 us at ~310 GB/s) -- below the PE stream, unlike the all-bf16
    version (7.65 MB ~ 25 us) which was DMA-bound.

Structure notes (inherited from the measured all-bf16 baseline):
  - Host packs chunk-tiled blocks so each dma_start moves one dense
    [128-partition x >=512B-contiguous-per-partition] block.
  - Column chunks <=510 (one fp32 PSUM bank); the two B chunks (split-
    expert piece) run first kt-outer, A chunks mt-outer with paired
    two-bank PSUM tiles drained by ONE strided DVE copy per pair.
  - Weights are per-(expert, kt) tiles [P, OUT]: the first real matmul
    only waits for w[B][kt=0] (0.13 MB) instead of a full expert (0.5 MB).
  - t DMAs ride the SP HWDGE ring; weight + most out DMAs ride ACT; a few
    outs ride SP to balance bytes (SP ~2.75 MB, ACT ~3.13 MB).
  - A short PE warm-up burst covers the head DMA window (HAM clock gate).
  - A deliberately small (30-slot) final chunk keeps the drain tail short.
"""

from math import ceil, sqrt

import ml_dtypes
import numpy as np

import concourse.bass as bass  # noqa: F401  (engine API namespace)
import concourse.mybir as mybir
import concourse.tile as tile
from concourse import bacc
from concourse.bass_utils import run_bass_kernel_spmd

B, Z, IN_DIM, OUT_DIM, R, M = 8192, 10, 512, 512, 8, 3
LORA_ALPHA = 8.0
SCALING = LORA_ALPHA / R
ALPHA = 1.0 / sqrt(IN_DIM)
N_CORES = 8
P = 128
KT = IN_DIM // P   # K tiles of the contraction dim
MT = OUT_DIM // P  # output-channel tiles
TSCALE = 2.0       # t pre-scale into e3m4 (folded back via 0.5x on W)
F32 = mybir.dt.float32
BF16 = mybir.dt.bfloat16
FP8E3 = mybir.dt.float8e3
NP_BF16 = ml_dtypes.bfloat16
NP_FP8 = ml_dtypes.float8_e3m4

N_WARM = 6

LAST_EXEC_NS = None
LAST_RESULTS = None

_program_cache: dict[tuple, object] = {}


def _seg_sizes(nslots_seg: int, tail_small: bool):
    """Split a segment into even slot counts <=170 (510 cols <= one fp32
    PSUM bank). With tail_small, end on a small 30-slot chunk so the
    drain tail (last PSUM copy + out DMA) is short."""
    if tail_small:
        sizes = []
        left = nslots_seg
        while left > 200:
            sizes.append(170)
            left -= 170
        if left > 60:
            sizes += [left - 30, 30]
        elif left:
            sizes.append(left)
        return sizes
    n = max(1, ceil(nslots_seg * 3 / 510))
    base = (nslots_seg // n) & ~1
    sizes = [base] * n
    rem = nslots_seg - base * n
    i = 0
    while rem > 0:
        sizes[i % n] += 2
        rem -= 2
        i += 1
    return sizes


def _chunk_plan(s1: int, s2: int):
    """Column chunks [(segment e, slot0, nslots)] covering both segments.

    Order: the B-segment (split-expert quarter) chunks first — small DMAs
    for a fast PE start, and they only need w[B], covering the w[A] load
    window — then the A chunks with a small final chunk for a short
    drain tail."""
    out = []
    s = 0
    for sz in _seg_sizes(s2, False):
        if sz:
            out.append((1, s, sz))
            s += sz
    s = 0
    for sz in _seg_sizes(s1, True):
        if sz:
            out.append((0, s, sz))
            s += sz
    return out


def _plan_offsets(s1: int, s2: int):
    """Returns (chunks, in_offs, out_offs, tin_len, tout_len)."""
    chunks = _chunk_plan(s1, s2)
    in_offs, out_offs = [], []
    oi = oo = 0
    for _, _, nslots in chunks:
        cols = nslots * 3
        in_offs.append(oi)
        out_offs.append(oo)
        oi += KT * cols
        oo += MT * cols
    return chunks, in_offs, out_offs, oi, oo


def _build_program(s1: int, s2: int, iters: int = 1, mode: str = "full"):
    """iters>1 repeats the whole body back-to-back — used only for slope
    timing; the graded path always uses iters=1. mode: full | pe_only
    (skip tin/out DMAs) | dma_only (skip matmuls+copies)."""
    chunks, in_offs, out_offs, tin_len, tout_len = _plan_offsets(s1, s2)
    do_dma = mode not in ("pe_only", "mm_only")
    do_pe = mode != "dma_only"
    do_copy = mode != "mm_only"

    nc = bacc.Bacc("TRN2", target_bir_lowering=False, debug=False,
                   num_devices=N_CORES)
    tk_d = nc.dram_tensor("tk", [P, tin_len], FP8E3, kind="ExternalInput")
    wt_d = nc.dram_tensor("wt", [2, KT, P, OUT_DIM], BF16,
                          kind="ExternalInput")
    out_d = nc.dram_tensor("out", [P, tout_len], BF16, kind="ExternalOutput")

    with tile.TileContext(nc) as tc:
        with (
            tc.tile_pool(name="wpool", bufs=2) as wpool,
            tc.tile_pool(name="warm", bufs=1) as warm_pool,
            tc.tile_pool(name="tpool", bufs=8) as tpool,
            tc.tile_pool(name="opool", bufs=5) as opool,
            tc.tile_pool(name="pmain", bufs=4, space="PSUM") as pm_pool,
        ):
          # PE warm-up: throwaway matmuls on a zeroed tile cover the head
          # DMA window (w[B][kt0] + tin chunk 0, ~0.6 us) so the PE queue
          # has work from t=0; more would delay the first real matmul.
          wz = warm_pool.tile([P, P], BF16, tag="wz", name="wz")
          nc.vector.memset(wz[:], 0.0)
          pw = pm_pool.tile([P, P], F32, tag="pm", name="pw")
          for i in range(N_WARM):
              nc.tensor.matmul(pw[:], wz[:], wz[:], start=True, stop=True)

          def _w_tiles(it):
              """Per-(expert, kt) weight tiles on the ACT ring; split-
              expert (B) tiles first, kt ascending — matches consumption
              order (B chunks run first, kt-outer)."""
              ws = {}
              for e in (chunks[0][0], 1 - chunks[0][0]):
                  for kt in range(KT):
                      w = wpool.tile([P, OUT_DIM], BF16, tag=f"w{e}_{kt}",
                                     name=f"w{e}_{kt}_{it}")
                      nc.scalar.dma_start(w[:], wt_d[e, kt])
                      ws[(e, kt)] = w
              return ws

          in_groups = [[ci] for ci in range(len(chunks))]
          chunk_grp = {}
          for gi, cis in enumerate(in_groups):
              off = 0
              for ci in cis:
                  chunk_grp[ci] = (gi, off)
                  off += chunks[ci][2] * 3

          def _gin_dma(gi, it, store):
              cis = in_groups[gi]
              gcols = sum(chunks[ci][2] * 3 for ci in cis)
              store[gi] = tpool.tile([P, KT * gcols], FP8E3, tag="tin",
                                     name=f"t_g{gi}_{it}")
              if do_dma:
                  io = in_offs[cis[0]]
                  nc.sync.dma_start(store[gi][:],
                                    tk_d[:, io:io + KT * gcols])
              else:
                  nc.sync.dma_start(store[gi][:, 0:2], tk_d[:, 0:2])

          w_next = None
          tin_next: dict = {}
          for it in range(iters):
            gtin = tin_next
            tin_next = {}

            # head order: tin-group0 (SP) || w[B] kt tiles (ACT)
            if 0 not in gtin:
                _gin_dma(0, it, gtin)
            w_sb = w_next if w_next is not None else _w_tiles(it)
            w_next = None
            if 1 not in gtin:
                _gin_dma(1, it, gtin)

            # ---- main: psum[mt] = sum_kt w[e,kt][:,mt].T @ tin[kt]
            for ci, (e, slot0, nslots) in enumerate(chunks):
                gi, coff = chunk_grp[ci]
                if gi not in gtin:
                    _gin_dma(gi)
                tin = gtin[gi]
                cols = nslots * 3
                ot = opool.tile([P, MT * cols], BF16, tag="ot",
                                name=f"o_{ci}_{it}")
                if not do_pe or not do_copy:
                    nc.vector.memset(ot[:, 0:2], 0.0)
                # paired two-bank PSUM tiles: mt pairs (0,1) and (2,3)
                # share a [P, 2, 512] tile so ONE strided DVE copy
                # drains both banks
                ps = [pm_pool.tile([P, 2, 512], F32, tag="pm",
                                   name=f"ps_{ci}_{h}_{it}")
                      for h in range(MT // 2)] if do_pe else []

                def _mm(mt, kt):
                    nc.tensor.matmul(
                        ps[mt // 2][:, mt % 2, 0:cols],
                        w_sb[(e, kt)][:, mt * P:(mt + 1) * P],
                        tin[:, coff * KT + kt * cols:
                            coff * KT + (kt + 1) * cols],
                        start=(kt == 0), stop=(kt == KT - 1))

                def _copy2(h):
                    base = 2 * h * cols
                    dst = ot[:, base:base + 2 * cols].rearrange(
                        "p (two c) -> p two c", two=2)
                    nc.vector.tensor_copy(dst, ps[h][:, :, 0:cols])

                if not do_pe:
                    pass
                elif ci < 2:
                    # kt-outer: the B chunks accumulate all four output
                    # tiles per arriving K-slice (first matmul only needs
                    # w[B][kt=0])
                    for kt in range(KT):
                        for mt in range(MT):
                            _mm(mt, kt)
                    if do_copy:
                        for h in range(MT // 2):
                            _copy2(h)
                else:
                    for mt in range(MT):
                        for kt in range(KT):
                            _mm(mt, kt)
                        if do_copy and mt % 2 == 1:
                            _copy2(mt // 2)
                if do_dma:
                    oo = out_offs[ci]
                    # ring balance: SP carries ~1.6MB of t; ACT ~1.05MB w
                    # + most of the ~3.2MB out. A few outs ride SP.
                    eng = (nc.sync if ci in (3, 5, len(chunks) - 1)
                           else nc.scalar)
                    eng.dma_start(out_d[:, oo:oo + MT * cols], ot[:])
                if ci == 3 and it + 1 < iters:
                    # prefetch next iteration's weights mid-iteration
                    # (wpool is double-buffered)
                    w_next = _w_tiles(it + 1)

    nc.compile()
    return nc


def _get_program(s1: int, s2: int, iters: int = 1, mode: str = "full"):
    key = (s1, s2, iters, mode)
    if key not in _program_cache:
        _program_cache[key] = _build_program(s1, s2, iters, mode)
    return _program_cache[key]


def _dense_fallback(t, node_attrs, weights, lora_A, lora_B):
    # Host-side general path: only reached if node_attrs is not one-hot
    # (never happens for this problem's setup_inputs).
    delta = np.einsum("zri,zor->zoi", lora_A, lora_B) * SCALING
    W = (weights + delta) * ALPHA
    out = np.zeros((B, OUT_DIM, M), np.float32)
    for z in range(Z):
        out += node_attrs[:, z, None, None] * np.matmul(W[z], t)
    return out


def _merged_weights(weights, lora_A, lora_B):
    """Host LoRA merge + scale + transpose + pack: [Z, KT, P, OUT] bf16.
    Includes the 1/TSCALE compensation for the fp8 t pre-scale."""
    delta = np.einsum("zor,zri->zoi", lora_B.astype(np.float32),
                      lora_A.astype(np.float32)) * np.float32(SCALING)
    W = (weights + delta) * np.float32(ALPHA / TSCALE)   # [Z, OUT, IN]
    return (
        W.transpose(0, 2, 1)                             # [Z, IN, OUT]
        .reshape(Z, KT, P, OUT_DIM).astype(NP_BF16)
    )


def _seg_nodes(nodes_by_z, eA, eB, s2, piece, seg):
    if seg == 0:
        return nodes_by_z[eA]
    return nodes_by_z[eB][piece * s2:(piece + 1) * s2]


def prepare(t, node_attrs, weights, lora_A, lora_B):
    """Host-side sharding: returns (s1, s2, in_maps, core_nodes) or None
    if the routing matrix is not one-hot (dense fallback needed)."""
    idx = node_attrs.argmax(axis=1)
    onehot = (np.count_nonzero(node_attrs, axis=1) == 1).all() and (
        node_attrs[np.arange(B), idx] == 1.0
    ).all()
    if not onehot:
        return None

    counts = np.bincount(idx, minlength=Z)
    order = np.argsort(counts, kind="stable")[::-1]
    bexp = order[:2].tolist()   # the two LARGEST experts, split in 4
    aexp = order[2:].tolist()   # eight whole experts, one per core
    s1 = (int(counts[aexp].max()) + 1) & ~1
    s2 = (ceil(int(counts[bexp].max()) / 4) + 1) & ~1
    s1 = max(s1, 64)
    s2 = max(s2, 32)
    chunks, in_offs, out_offs, tin_len, _ = _plan_offsets(s1, s2)
    nodes_by_z = [np.where(idx == z)[0] for z in range(Z)]

    t_fp8 = (t * np.float32(TSCALE)).astype(NP_FP8)  # [B, IN, M]
    wt_all = _merged_weights(weights, lora_A, lora_B)

    in_maps = []
    core_nodes = []
    for k in range(N_CORES):
        eA = aexp[k]
        eB = bexp[0] if k < 4 else bexp[1]
        piece = k % 4
        tk = np.zeros((P, tin_len), NP_FP8)
        for ci, (seg, slot0, nslots) in enumerate(chunks):
            seg_n = _seg_nodes(nodes_by_z, eA, eB, s2, piece, seg)
            sl = seg_n[slot0:slot0 + nslots]
            ns = len(sl)
            if ns == 0:
                continue
            cols, ca = nslots * 3, ns * 3
            # [ns, IN, 3] -> [IN, ca] -> [kt, p, ca] -> [p, kt, cols]
            A = t_fp8[sl].transpose(1, 0, 2).reshape(IN_DIM, ca)
            blk = np.zeros((P, KT, cols), NP_FP8)
            blk[:, :, :ca] = A.reshape(KT, P, ca).transpose(1, 0, 2)
            io = in_offs[ci]
            tk[:, io:io + KT * cols] = blk.reshape(P, KT * cols)
        in_maps.append({
            "tk": tk,
            "wt": np.ascontiguousarray(wt_all[[eA, eB]]),
        })
        core_nodes.append((eA, eB, piece))
    return s1, s2, in_maps, core_nodes


def assemble(s1, s2, core_nodes, results, nodes_by_z):
    chunks, _, out_offs, _, _ = _plan_offsets(s1, s2)
    out_full = np.zeros((B, OUT_DIM, M), np.float32)
    for k in range(N_CORES):
        eA, eB, piece = core_nodes[k]
        o = results[k]["out"]
        for ci, (seg, slot0, nslots) in enumerate(chunks):
            seg_n = _seg_nodes(nodes_by_z, eA, eB, s2, piece, seg)
            sl = seg_n[slot0:slot0 + nslots]
            ns = len(sl)
            if ns == 0:
                continue
            cols, ca = nslots * 3, ns * 3
            oo = out_offs[ci]
            blk = o[:, oo:oo + MT * cols].reshape(P, MT, cols)[:, :, :ca]
            # [p, mt, ca] -> [mt, p, ca] -> [OUT, ns, 3] -> [ns, OUT, 3]
            out_full[sl] = (
                blk.transpose(1, 0, 2).reshape(OUT_DIM, ns, M)
                .transpose(1, 0, 2).astype(np.float32)
            )
    return out_full


def kernel(t, node_attrs, weights, lora_A, lora_B):
    global LAST_EXEC_NS, LAST_RESULTS
    t = np.ascontiguousarray(t, dtype=np.float32)
    node_attrs = np.asarray(node_attrs, dtype=np.float32)
    weights = np.asarray(weights, dtype=np.float32)
    lora_A = np.ascontiguousarray(lora_A, dtype=np.float32)
    lora_B = np.asarray(lora_B, dtype=np.float32)

    prep = prepare(t, node_attrs, weights, lora_A, lora_B)
    if prep is None:
        return _dense_fallback(t, node_attrs, weights, lora_A, lora_B)
    s1, s2, in_maps, core_nodes = prep
    idx = node_attrs.argmax(axis=1)
    nodes_by_z = [np.where(idx == z)[0] for z in range(Z)]

    nc = _get_program(s1, s2)
    res = run_bass_kernel_spmd(nc, in_maps, list(range(N_CORES)))
    LAST_EXEC_NS = res.exec_time_ns
    LAST_RESULTS = res
    return assemble(s1, s2, core_nodes, res.results, nodes_by_z)


# revision 8
# speedup vs baseline: 1.0406x; 1.0406x over previous
"""Trainium2 Bass kernel for nn_LoRAElementLinear (MoE-routed per-node linear).

Math (reference):
    delta_w[z] = lora_A[z] contracted with lora_B[z] * SCALING         # [OUT, IN]
    W[z]       = (weights[z] + delta_w[z]) * ALPHA                     # [OUT, IN]
    out[b]     = sum_z node_attrs[b, z] * (W[z] @ t[b])                # [OUT, M]

node_attrs is a one-hot expert indicator (moe_routing), so out[b] = W[expert(b)] @ t[b].
The LoRA merge is folded into W on the host; the device runs only the routed
batched matmul.

Numerics: t is shipped as fp8 e3m4 (x2 pre-scale, folded back out through a
0.5x on W), W as bf16, PSUM accumulates fp32, output bf16. Measured on the
real data (host fp64 sim): rel err 1.30e-2 vs the 2e-2 gate. e4m3 anywhere
measures 2.6-3.9e-2 (fails), both-e3m4 measures 1.96e-2 (too close), so
DoubleRow fp8 (e4m3-only) is unusable and the PE runs at the bf16 rate of
1 col/cycle; fp8 on t is purely a DMA-bytes win (t is the largest input).

Sharding (host side): group nodes by expert. The two LARGEST experts are
split into 4 quarter-pieces each (cores 0-3 carry quarters of the largest,
cores 4-7 of the second); the remaining eight experts go whole, one per
core. Per-core profile [s1=max(whole counts), s2=max quarter] is SPMD-
uniform. On the seed-0 data: s1=828, s2=220 -> 1048 slots/core (2.3% pad)
vs the old split-the-two-smallest scheme's 1100 (7.4% pad).

Per-core streams (s1=828, s2=220 -> 3144 cols), all HW-measured via the
R64/R128 unrolled-body difference method on these axon-tunneled cores:
  - PE: 16 passes x 3144 cols = 50.3k cycles. Under sustained 8-core load
    this chip holds ~1.94-2.0 GHz effective (P0 power state, not the
    2.4 GHz datasheet clock; a bare MM-stream probe shows the same rate),
    so the PE stream is ~25.2-25.9 us. mm_only measures 25.9 us.
  - DMA: t fp8 1.61 MB + W bf16 1.05 MB + out bf16 3.22 MB = 5.9 MB;
    dma_only measures 20.3 us -- comfortably below the PE stream, unlike
    the all-bf16 baseline (7.65 MB, DMA-bound).
  - full measures ~25.8 us vs the bf16 baseline's 27.0-27.4 us under
    identical conditions (same-session A/B): the win is 4.7% fewer PE
    cycles (less padding) plus the removed DMA bottleneck. full ==
    mm_only within noise, i.e. the kernel sits on the PE roofline for
    this clock state; sim (TimelineSim) and cycle model agree the
    remaining non-PE overhead is <1 us.

Structure notes (inherited from the measured all-bf16 baseline):
  - Host packs chunk-tiled blocks so each dma_start moves one dense
    [128-partition x >=512B-contiguous-per-partition] block.
  - Column chunks <=510 (one fp32 PSUM bank); the two B chunks (split-
    expert piece) run first kt-outer, A chunks mt-outer with paired
    two-bank PSUM tiles drained by ONE strided DVE copy per pair.
  - Weights are per-(expert, kt) tiles [P, OUT]: the first real matmul
    only waits for w[B][kt=0] (0.13 MB) instead of a full expert (0.5 MB).
  - t DMAs ride the SP HWDGE ring; weight + most out DMAs ride ACT; a few
    outs ride SP to balance bytes (SP ~2.75 MB, ACT ~3.13 MB).
  - A short PE warm-up burst covers the head DMA window (HAM clock gate).
  - A deliberately small (30-slot) final chunk keeps the drain tail short.
"""

from math import ceil, sqrt

import ml_dtypes
import numpy as np

import concourse.bass as bass  # noqa: F401  (engine API namespace)
import concourse.mybir as mybir
import concourse.tile as tile
from concourse import bacc
from concourse.bass_utils import run_bass_kernel_spmd

B, Z, IN_DIM, OUT_DIM, R, M = 8192, 10, 512, 512, 8, 3
LORA_ALPHA = 8.0
SCALING = LORA_ALPHA / R
ALPHA = 1.0 / sqrt(IN_DIM)
N_CORES = 8
P = 128
KT = IN_DIM // P   # K tiles of the contraction dim
MT = OUT_DIM // P  # output-channel tiles
TSCALE = 2.0       # t pre-scale into e3m4 (folded back via 0.5x on W)
F32 = mybir.dt.float32
BF16 = mybir.dt.bfloat16
FP8E3 = mybir.dt.float8e3
NP_BF16 = ml_dtypes.bfloat16
NP_FP8 = ml_dtypes.float8_e3m4

N_WARM = 6

LAST_EXEC_NS = None
LAST_RESULTS = None

_program_cache: dict[tuple, object] = {}


def _seg_sizes(nslots_seg: int, tail_small: bool):
    """Split a segment into even slot counts <=170 (510 cols <= one fp32
    PSUM bank). With tail_small, end on a small 30-slot chunk so the
    drain tail (last PSUM copy + out DMA) is short."""
    if tail_small:
        sizes = []
        left = nslots_seg
        while left > 200:
            sizes.append(170)
            left -= 170
        if left > 60:
            sizes += [left - 30, 30]
        elif left:
            sizes.append(left)
        return sizes
    n = max(1, ceil(nslots_seg * 3 / 510))
    base = (nslots_seg // n) & ~1
    sizes = [base] * n
    rem = nslots_seg - base * n
    i = 0
    while rem > 0:
        sizes[i % n] += 2
        rem -= 2
        i += 1
    return sizes


def _chunk_plan(s1: int, s2: int):
    """Column chunks [(segment e, slot0, nslots)] covering both segments.

    Order: the B-segment (split-expert quarter) chunks first — small DMAs
    for a fast PE start, and they only need w[B], covering the w[A] load
    window — then the A chunks with a small final chunk for a short
    drain tail."""
    out = []
    s = 0
    for sz in _seg_sizes(s2, False):
        if sz:
            out.append((1, s, sz))
            s += sz
    s = 0
    for sz in _seg_sizes(s1, True):
        if sz:
            out.append((0, s, sz))
            s += sz
    return out


def _plan_offsets(s1: int, s2: int):
    """Returns (chunks, in_offs, out_offs, tin_len, tout_len)."""
    chunks = _chunk_plan(s1, s2)
    in_offs, out_offs = [], []
    oi = oo = 0
    for _, _, nslots in chunks:
        cols = nslots * 3
        in_offs.append(oi)
        out_offs.append(oo)
        oi += KT * cols
        oo += MT * cols
    return chunks, in_offs, out_offs, oi, oo


def _build_program(s1: int, s2: int, iters: int = 1, mode: str = "full"):
    """iters>1 repeats the whole body back-to-back — used only for slope
    timing; the graded path always uses iters=1. mode: full | pe_only
    (skip tin/out DMAs) | dma_only (skip matmuls+copies)."""
    chunks, in_offs, out_offs, tin_len, tout_len = _plan_offsets(s1, s2)
    do_dma = mode not in ("pe_only", "mm_only", "mm_bf16")
    do_pe = mode != "dma_only"
    do_copy = mode not in ("mm_only", "mm_bf16")
    tin_dt = BF16 if mode == "mm_bf16" else FP8E3

    nc = bacc.Bacc("TRN2", target_bir_lowering=False, debug=False,
                   num_devices=N_CORES)
    tk_d = nc.dram_tensor("tk", [P, tin_len], FP8E3, kind="ExternalInput")
    wt_d = nc.dram_tensor("wt", [2, KT, P, OUT_DIM], BF16,
                          kind="ExternalInput")
    out_d = nc.dram_tensor("out", [P, tout_len], BF16, kind="ExternalOutput")

    with tile.TileContext(nc) as tc:
        with (
            tc.tile_pool(name="wpool", bufs=2) as wpool,
            tc.tile_pool(name="warm", bufs=1) as warm_pool,
            tc.tile_pool(name="tpool", bufs=8) as tpool,
            tc.tile_pool(name="opool", bufs=5) as opool,
            tc.tile_pool(name="pmain", bufs=4, space="PSUM") as pm_pool,
        ):
          # PE warm-up: throwaway matmuls on a zeroed tile cover the head
          # DMA window (w[B][kt0] + tin chunk 0, ~0.6 us) so the PE queue
          # has work from t=0; more would delay the first real matmul.
          wz = warm_pool.tile([P, P], BF16, tag="wz", name="wz")
          nc.vector.memset(wz[:], 0.0)
          pw = pm_pool.tile([P, P], F32, tag="pm", name="pw")
          for i in range(N_WARM):
              nc.tensor.matmul(pw[:], wz[:], wz[:], start=True, stop=True)

          def _w_tiles(it):
              """Per-(expert, kt) weight tiles on the ACT ring; split-
              expert (B) tiles first, kt ascending — matches consumption
              order (B chunks run first, kt-outer)."""
              ws = {}
              for e in (chunks[0][0], 1 - chunks[0][0]):
                  for kt in range(KT):
                      w = wpool.tile([P, OUT_DIM], BF16, tag=f"w{e}_{kt}",
                                     name=f"w{e}_{kt}_{it}")
                      nc.scalar.dma_start(w[:], wt_d[e, kt])
                      ws[(e, kt)] = w
              return ws

          in_groups = [[ci] for ci in range(len(chunks))]
          chunk_grp = {}
          for gi, cis in enumerate(in_groups):
              off = 0
              for ci in cis:
                  chunk_grp[ci] = (gi, off)
                  off += chunks[ci][2] * 3

          def _gin_dma(gi, it, store):
              cis = in_groups[gi]
              gcols = sum(chunks[ci][2] * 3 for ci in cis)
              store[gi] = tpool.tile([P, KT * gcols], tin_dt, tag="tin",
                                     name=f"t_g{gi}_{it}")
              if do_dma:
                  io = in_offs[cis[0]]
                  nc.sync.dma_start(store[gi][:],
                                    tk_d[:, io:io + KT * gcols])
              elif tin_dt is FP8E3:
                  nc.sync.dma_start(store[gi][:, 0:2], tk_d[:, 0:2])
              else:
                  nc.vector.memset(store[gi][:, 0:2], 0.0)

          w_next = None
          tin_next: dict = {}
          for it in range(iters):
            gtin = tin_next
            tin_next = {}

            # head order: tin-group0 (SP) || w[B] kt tiles (ACT)
            if 0 not in gtin:
                _gin_dma(0, it, gtin)
            w_sb = w_next if w_next is not None else _w_tiles(it)
            w_next = None
            if 1 not in gtin:
                _gin_dma(1, it, gtin)

            # ---- main: psum[mt] = sum_kt w[e,kt][:,mt].T @ tin[kt]
            for ci, (e, slot0, nslots) in enumerate(chunks):
                gi, coff = chunk_grp[ci]
                if gi not in gtin:
                    _gin_dma(gi, it, gtin)
                tin = gtin[gi]
                cols = nslots * 3
                ot = opool.tile([P, MT * cols], BF16, tag="ot",
                                name=f"o_{ci}_{it}")
                if not do_pe or not do_copy:
                    nc.vector.memset(ot[:, 0:2], 0.0)
                # paired two-bank PSUM tiles: mt pairs (0,1) and (2,3)
                # share a [P, 2, 512] tile so ONE strided DVE copy
                # drains both banks
                ps = [pm_pool.tile([P, 2, 512], F32, tag="pm",
                                   name=f"ps_{ci}_{h}_{it}")
                      for h in range(MT // 2)] if do_pe else []

                def _mm(mt, kt):
                    nc.tensor.matmul(
                        ps[mt // 2][:, mt % 2, 0:cols],
                        w_sb[(e, kt)][:, mt * P:(mt + 1) * P],
                        tin[:, coff * KT + kt * cols:
                            coff * KT + (kt + 1) * cols],
                        start=(kt == 0), stop=(kt == KT - 1))

                def _copy2(h):
                    base = 2 * h * cols
                    dst = ot[:, base:base + 2 * cols].rearrange(
                        "p (two c) -> p two c", two=2)
                    nc.vector.tensor_copy(dst, ps[h][:, :, 0:cols])

                if not do_pe:
                    pass
                elif ci < 2:
                    # kt-outer: the B chunks accumulate all four output
                    # tiles per arriving K-slice (first matmul only needs
                    # w[B][kt=0])
                    for kt in range(KT):
                        for mt in range(MT):
                            _mm(mt, kt)
                    if do_copy:
                        for h in range(MT // 2):
                            _copy2(h)
                else:
                    for mt in range(MT):
                        for kt in range(KT):
                            _mm(mt, kt)
                        if do_copy and mt % 2 == 1:
                            _copy2(mt // 2)
                if do_dma:
                    oo = out_offs[ci]
                    # ring balance: SP carries ~1.6MB of t; ACT ~1.05MB w
                    # + most of the ~3.2MB out. A few outs ride SP.
                    eng = (nc.sync if ci in (3, 5, len(chunks) - 1)
                           else nc.scalar)
                    eng.dma_start(out_d[:, oo:oo + MT * cols], ot[:])
                if ci == 3 and it + 1 < iters:
                    # prefetch next iteration's weights mid-iteration
                    # (wpool is double-buffered)
                    w_next = _w_tiles(it + 1)
                if ci == 5 and it + 1 < iters:
                    # prefetch next iteration's first two t chunks so the
                    # PE stream has no tin-wait at the iteration boundary
                    _gin_dma(0, it + 1, tin_next)
                    _gin_dma(1, it + 1, tin_next)

    nc.compile()
    return nc


def _get_program(s1: int, s2: int, iters: int = 1, mode: str = "full"):
    key = (s1, s2, iters, mode)
    if key not in _program_cache:
        _program_cache[key] = _build_program(s1, s2, iters, mode)
    return _program_cache[key]


def _dense_fallback(t, node_attrs, weights, lora_A, lora_B):
    # Host-side general path: only reached if node_attrs is not one-hot
    # (never happens for this problem's setup_inputs).
    delta = np.einsum("zri,zor->zoi", lora_A, lora_B) * SCALING
    W = (weights + delta) * ALPHA
    out = np.zeros((B, OUT_DIM, M), np.float32)
    for z in range(Z):
        out += node_attrs[:, z, None, None] * np.matmul(W[z], t)
    return out


def _merged_weights(weights, lora_A, lora_B):
    """Host LoRA merge + scale + transpose + pack: [Z, KT, P, OUT] bf16.
    Includes the 1/TSCALE compensation for the fp8 t pre-scale."""
    delta = np.einsum("zor,zri->zoi", lora_B.astype(np.float32),
                      lora_A.astype(np.float32)) * np.float32(SCALING)
    W = (weights + delta) * np.float32(ALPHA / TSCALE)   # [Z, OUT, IN]
    return (
        W.transpose(0, 2, 1)                             # [Z, IN, OUT]
        .reshape(Z, KT, P, OUT_DIM).astype(NP_BF16)
    )


def _seg_nodes(nodes_by_z, eA, eB, s2, piece, seg):
    if seg == 0:
        return nodes_by_z[eA]
    return nodes_by_z[eB][piece * s2:(piece + 1) * s2]


def prepare(t, node_attrs, weights, lora_A, lora_B):
    """Host-side sharding: returns (s1, s2, in_maps, core_nodes) or None
    if the routing matrix is not one-hot (dense fallback needed)."""
    idx = node_attrs.argmax(axis=1)
    onehot = (np.count_nonzero(node_attrs, axis=1) == 1).all() and (
        node_attrs[np.arange(B), idx] == 1.0
    ).all()
    if not onehot:
        return None

    counts = np.bincount(idx, minlength=Z)
    order = np.argsort(counts, kind="stable")[::-1]
    bexp = order[:2].tolist()   # the two LARGEST experts, split in 4
    aexp = order[2:].tolist()   # eight whole experts, one per core
    s1 = (int(counts[aexp].max()) + 1) & ~1
    s2 = (ceil(int(counts[bexp].max()) / 4) + 1) & ~1
    s1 = max(s1, 64)
    s2 = max(s2, 32)
    chunks, in_offs, out_offs, tin_len, _ = _plan_offsets(s1, s2)
    nodes_by_z = [np.where(idx == z)[0] for z in range(Z)]

    t_fp8 = (t * np.float32(TSCALE)).astype(NP_FP8)  # [B, IN, M]
    wt_all = _merged_weights(weights, lora_A, lora_B)

    in_maps = []
    core_nodes = []
    for k in range(N_CORES):
        eA = aexp[k]
        eB = bexp[0] if k < 4 else bexp[1]
        piece = k % 4
        tk = np.zeros((P, tin_len), NP_FP8)
        for ci, (seg, slot0, nslots) in enumerate(chunks):
            seg_n = _seg_nodes(nodes_by_z, eA, eB, s2, piece, seg)
            sl = seg_n[slot0:slot0 + nslots]
            ns = len(sl)
            if ns == 0:
                continue
            cols, ca = nslots * 3, ns * 3
            # [ns, IN, 3] -> [IN, ca] -> [kt, p, ca] -> [p, kt, cols]
            A = t_fp8[sl].transpose(1, 0, 2).reshape(IN_DIM, ca)
            blk = np.zeros((P, KT, cols), NP_FP8)
            blk[:, :, :ca] = A.reshape(KT, P, ca).transpose(1, 0, 2)
            io = in_offs[ci]
            tk[:, io:io + KT * cols] = blk.reshape(P, KT * cols)
        in_maps.append({
            "tk": tk,
            "wt": np.ascontiguousarray(wt_all[[eA, eB]]),
        })
        core_nodes.append((eA, eB, piece))
    return s1, s2, in_maps, core_nodes


def assemble(s1, s2, core_nodes, results, nodes_by_z):
    chunks, _, out_offs, _, _ = _plan_offsets(s1, s2)
    out_full = np.zeros((B, OUT_DIM, M), np.float32)
    for k in range(N_CORES):
        eA, eB, piece = core_nodes[k]
        o = results[k]["out"]
        for ci, (seg, slot0, nslots) in enumerate(chunks):
            seg_n = _seg_nodes(nodes_by_z, eA, eB, s2, piece, seg)
            sl = seg_n[slot0:slot0 + nslots]
            ns = len(sl)
            if ns == 0:
                continue
            cols, ca = nslots * 3, ns * 3
            oo = out_offs[ci]
            blk = o[:, oo:oo + MT * cols].reshape(P, MT, cols)[:, :, :ca]
            # [p, mt, ca] -> [mt, p, ca] -> [OUT, ns, 3] -> [ns, OUT, 3]
            out_full[sl] = (
                blk.transpose(1, 0, 2).reshape(OUT_DIM, ns, M)
                .transpose(1, 0, 2).astype(np.float32)
            )
    return out_full


def kernel(t, node_attrs, weights, lora_A, lora_B):
    global LAST_EXEC_NS, LAST_RESULTS
    t = np.ascontiguousarray(t, dtype=np.float32)
    node_attrs = np.asarray(node_attrs, dtype=np.float32)
    weights = np.asarray(weights, dtype=np.float32)
    lora_A = np.ascontiguousarray(lora_A, dtype=np.float32)
    lora_B = np.asarray(lora_B, dtype=np.float32)

    prep = prepare(t, node_attrs, weights, lora_A, lora_B)
    if prep is None:
        return _dense_fallback(t, node_attrs, weights, lora_A, lora_B)
    s1, s2, in_maps, core_nodes = prep
    idx = node_attrs.argmax(axis=1)
    nodes_by_z = [np.where(idx == z)[0] for z in range(Z)]

    nc = _get_program(s1, s2)
    res = run_bass_kernel_spmd(nc, in_maps, list(range(N_CORES)))
    LAST_EXEC_NS = res.exec_time_ns
    LAST_RESULTS = res
    return assemble(s1, s2, core_nodes, res.results, nodes_by_z)
